# revision 37
# baseline (speedup 1.0000x reference)
"""CrossStreamAttention Trainium2 kernel v3 (8-core SPMD, data-parallel over
query rows) — algebra-refactored, fp8 DoubleRow everywhere, fused pipeline.

Reference (fp32):
    q = x_q @ Wq.T + bq; k = x_kv @ Wk.T + bk; v = x_kv @ Wv.T + bv
    out = softmax(q @ k.T / 16) @ v          (N = M = 8192, D = 256)

Algebraic refactor (removes the k and v projections entirely):
    S = q @ k.T = x_q @ (Wq.T @ Wk) @ x_kv.T + (bq.T Wk) @ x_kv.T + const(n)
      -> q2 = x_q @ A + u,  A = Wq.T Wk,  u = Wk.T bq   (const(n) cancels in
         softmax over m; bk enters only via const(n) so it is never needed)
    P = exp(S/16);  out = (P @ x_kv) @ Wv.T / Z + bv,  Z = P @ 1

v3 structure (all phases fused into one pipeline; ACT exp stream is the
binding resource at ~55us busy + overheads, PE ~60us):
  - prefix (under the x_kv DMA shadow): A/u (fp32 PE), x_q^T, q2 = A-proj
    (+u via K=1 ones matmul) stored fp8 (q28), Wv^T last
  - per kv-chunk loop (64 chunks of 128):
      * per pair (one-pair lookahead): fp32 PE-transpose group straight
        from the DMA staging tiles; DVE copy-out casts to fp8 xkvT8 [d, m]
      * pass1: S^T chunk = one DR matmul per n-half (xkvT8 [128,2,128] fp8
        stationary x q28 [128,2,512] fp8 moving, K=256 DoubleRow)
      * exp on ACT -> p8 (e4m3), materialized in SBUF
      * per pair: O^T(hh0) DR accumulation (xkv8 stationary) + Z for both
        halves into ONE psum bank ([2, NH]: interleaved accumulation chains,
        hh1 via a 2-column 0/1-selector stationary, no start= after pair 0)
    psum budget in the loop: tp 1 + st 2x2 + o(hh0) 2 + z 1 = 8 banks.
  - tail: O^T(hh1) DR pass with the reciprocal chain (zps -> SBUF -> DMA
    partition-merge -> PE column transposes -> 1/Z) and the first four
    epilogue n-chunks interleaved into it; then the remaining chunks of
    out = (O^T.T @ Wv^T + Z*bv) / Z, streamed out.

Measured (test.py slope method): v2 baseline 172us -> v3 104us, rel err
1.80e-2 (gate 2e-2). Tried and rejected: bf16 DMA-transpose for x_kv
(228us — per-instruction DMA overheads), dropping the PE transposes
entirely (no gain — PE is not binding), gpsimd partition_all_reduce for Z
(reads+writes 128x the data), e3m4 anywhere (DoubleRow is e4m3/e5m2 only).

Precision: S error from fp8 x_kv AND fp8 q2 (the p8 e4m3 storage error
dominates); P and the P@V path are e4m3 (denominator uses the same quantized
P, so errors partially cancel). Gate 2e-2.
"""

import sys

for _p in ("/opt/trn_rl_repo", "/root/.axon_site/_ro/trn_rl_repo"):
    if _p not in sys.path:
        sys.path.append(_p)

import numpy as np

import concourse.bass as bass
import concourse.mybir as mybir
import concourse.tile as tile
from concourse import bacc
from concourse.masks import make_identity

N, M, D = 8192, 8192, 256
NCORES = 8
NL = N // NCORES          # query rows per core (1024)
P = 128                   # partition dim
KD = D // P               # 2 d-tiles of 128
MT = M // P               # 64 kv chunks of 128
NPAIR = MT // 2           # 32 chunk pairs (DoubleRow K=256)
NH = 512                  # n-half
NCH = NL // P             # 8 n-chunks
SCALE = 1.0 / 16.0        # 1/sqrt(D)

FP32 = mybir.dt.float32
F32R = mybir.dt.float32r
FP8 = mybir.dt.float8e4
BF16 = mybir.dt.bfloat16
AF = mybir.ActivationFunctionType
DR = mybir.MatmulPerfMode.DoubleRow


def _build_nc(reps=1, muls=()):
    """muls knobs: cut -> keep phases 0..cut (2=stop after loop/p8,
    3=+O/Z, 99=full); salt -> cache-bust; q16 -> bf16 q2 fallback (no DR
    pass1); zsplit -> Z in two psum banks instead of the one-bank trick;
    dmat -> x_kv transposes via bf16 DMA-transpose instead of PE."""
    muls = dict(muls)
    nc = bacc.Bacc("TRN2", target_bir_lowering=False, debug=False,
                   num_devices=NCORES)

    xq_d = nc.dram_tensor("x_q", [NL, D], FP32, kind="ExternalInput")
    xkv_d = nc.dram_tensor("x_kv", [M, D], FP32, kind="ExternalInput")
    wq_d = nc.dram_tensor("Wq", [D, D], FP32, kind="ExternalInput")
    wk_d = nc.dram_tensor("Wk", [D, D], FP32, kind="ExternalInput")
    wv_d = nc.dram_tensor("Wv", [D, D], FP32, kind="ExternalInput")
    bq_d = nc.dram_tensor("bq", [KD, P, 1], FP32, kind="ExternalInput")
    bv_d = nc.dram_tensor("bv", [1, D], FP32, kind="ExternalInput")
    out_d = nc.dram_tensor("out", [NL, D], FP32, kind="ExternalOutput")

    with tile.TileContext(nc) as tc:
        for rep in range(reps):
            _body(tc, xq_d, xkv_d, wq_d, wk_d, wv_d, bq_d, bv_d, out_d, rep,
                  muls)
    nc.compile()
    return nc


def _emit_probes(tc, fin, probes, out_d):
    """Tiny consumer chain so walrus cannot dead-code-eliminate cut phases."""
    nc = tc.nc
    npb = len(probes)
    pb = fin.tile([P, max(npb, 1)], FP32, tag="pb", name="pb")
    for i, ap in enumerate(probes):
        ps = ap.shape[0]
        nc.vector.tensor_copy(pb[0:ps, i:i + 1], ap)
    nc.sync.dma_start(out_d[0:P, 0:npb], pb[:, 0:npb])


def _body(tc, xq_d, xkv_d, wq_d, wk_d, wv_d, bq_d, bv_d, out_d, rep, muls={}):
    nc = tc.nc
    mult = mybir.AluOpType.mult
    cut = muls.get("cut", 99)
    salt = muls.get("salt", 0)
    q16 = muls.get("q16", 0)       # 1: bf16 q2 fallback (pass1 without DR)
    zsplit = muls.get("zsplit", 0)  # 1: Z in 2 banks (no one-bank trick)
    dmat = muls.get("dmat", 0)     # 1: x_kv transpose via bf16 DMA-transpose
    notr = muls.get("notr", 0)     # 1: skip transposes (timing probe only)
    probes = []

    with tc.tile_pool(name=f"c{rep}s{salt}", bufs=1) as cpool, \
         tc.tile_pool(name=f"xs{rep}", bufs=4) as xst, \
         tc.tile_pool(name=f"fin{rep}", bufs=2) as fin:
        # prefix-only SBUF (weights, x_q staging/transpose) — released before
        # the big p8 pool opens so the space is reused
        pwp = tc.alloc_tile_pool(name=f"pw{rep}", bufs=1)

        # ---------------- constants + persistent tiles ----------------
        ident = cpool.tile([P, P], FP32, tag="ident", name="ident")
        make_identity(nc, ident[:])
        id8 = cpool.tile([P, P], FP8, tag="id8", name="id8")
        nc.gpsimd.tensor_copy(id8[:], ident[:])
        ones32f = cpool.tile([P, KD, 16], FP32, tag="ones32f", name="ones32f")
        nc.gpsimd.memset(ones32f[:], 1.0)
        ones8 = cpool.tile([P, KD, 16], FP8, tag="ones8", name="ones8")
        nc.gpsimd.tensor_copy(ones8[:], ones32f[:])
        # Z selector for the hh1 chain: 2 columns [0, 1] (writes psum
        # partitions {0,1}; partition 0 accumulates zeros)
        zsel32 = cpool.tile([P, KD, 16], FP32, tag="zsel32", name="zsel32")
        nc.gpsimd.memset(zsel32[:], 0.0)
        nc.gpsimd.memset(zsel32[:, :, 3:4], 1.0)
        zsel8 = cpool.tile([P, KD, 16], FP8, tag="zsel8", name="zsel8")
        nc.gpsimd.tensor_copy(zsel8[:], zsel32[:])
        onesr = cpool.tile([1, NH], F32R, tag="onesr", name="onesr")
        nc.gpsimd.memset(onesr[:].bitcast(FP32), 1.0)

        # prefix-critical DMAs first (wq/wk for A, x_q for the transposes);
        # bq/bv/wv land later — bq only feeds u, bv/wv only the epilogue
        wq_sb = pwp.tile([P, KD, D], FP32, tag="wq", name="wq_sb")
        wk_sb = pwp.tile([P, KD, D], FP32, tag="wk", name="wk_sb")
        wv_sb = pwp.tile([P, KD, D], FP32, tag="wv", name="wv_sb")
        nc.sync.dma_start(wq_sb[:], wq_d.rearrange("(t p) d -> p t d", p=P))
        nc.sync.dma_start(wk_sb[:], wk_d.rearrange("(t p) d -> p t d", p=P))

        bq_sb = cpool.tile([P, KD, 1], FP32, tag="bq", name="bq_sb")
        nc.sync.dma_start(bq_sb[:], bq_d.rearrange("a p 1 -> p a 1"))

        xkvT8 = cpool.tile([P, KD, M], FP8, tag="xkvT8", name="xkvT8")
        if notr:
            nc.gpsimd.memset(xkvT8[:], 0.25)
        if q16:
            q2x = cpool.tile([P, KD, NL], BF16, tag="q2b", name="q2b")
        else:
            q2x = cpool.tile([P, KD, NL], FP8, tag="q28", name="q28")
        A_sb = cpool.tile([P, KD, D], F32R, tag="A", name="A_sb")
        u_row = cpool.tile([1, D], F32R, tag="u", name="u_row")
        WvT = cpool.tile([P, KD, D], F32R, tag="WvT", name="WvT")
        xqT = pwp.tile([P, KD, NL], F32R, tag="xqT", name="xqT")
        xkv8 = cpool.tile([P, MT, D], FP8, tag="xkv8", name="xkv8")
        OT_sb = cpool.tile([P, KD, NL], F32R, tag="OT", name="OT_sb")
        Z2 = cpool.tile([2, NL // 2], FP32, tag="Z2", name="Z2")
        Zr = cpool.tile([1, NL], F32R, tag="Zr", name="Zr")
        Z32 = cpool.tile([1, NL], FP32, tag="Z32", name="Z32")
        rec_sb = cpool.tile([P, NCH], FP32, tag="rec", name="rec_sb")
        warm8 = cpool.tile([P, 16], FP8, tag="warm8", name="warm8")

        # warm the ACT exp table off the critical path
        nc.scalar.activation(warm8[:, 0:1], bq_sb[:, 0, :], AF.Exp, scale=1.0)

        # x_q in two halves: the n 0..511 half unblocks the hh0 q2 chain
        # (and with it the first exps) before the second half lands
        xqs = pwp.tile([P, NCH, D], FP32, tag="xqs", name="xqs")
        nc.sync.dma_start(xqs[:, 0:NCH // 2, :],
                          xq_d[0:NL // 2].rearrange("(t p) d -> p t d", p=P))

        # ---------------- x_kv DMA + fp8 convert (Pool, off PE path) -------
        GRP = 8                    # chunks per DMA/convert group
        if dmat:
            xkv16 = cpool.tile([P, MT, D], BF16, tag="xkv16", name="xkv16")
        xst_tiles = []

        def emit_xkv_group(g):
            xs = xst.tile([P, GRP, D], FP32, tag="xs", name="xs")
            xst_tiles.append(xs)
            nc.sync.dma_start(
                xs[:],
                xkv_d[g * GRP * P:(g + 1) * GRP * P, :]
                .rearrange("(t p) d -> p t d", p=P))
            if dmat:
                nc.gpsimd.tensor_copy(xkv16[:, g * GRP:(g + 1) * GRP, :],
                                      xs[:])
                nc.vector.tensor_copy(xkv8[:, g * GRP:(g + 1) * GRP, :],
                                      xkv16[:, g * GRP:(g + 1) * GRP, :])
            else:
                nc.gpsimd.tensor_copy(xkv8[:, g * GRP:(g + 1) * GRP, :],
                                      xs[:])

        # queue order: x_kv g0 right after the first x_q half, then the
        # remaining prefix inputs, then the bulk of x_kv
        emit_xkv_group(0)
        nc.sync.dma_start(xqs[:, NCH // 2:NCH, :],
                          xq_d[NL // 2:NL].rearrange("(t p) d -> p t d", p=P))
        bv_row = cpool.tile([1, D], FP32, tag="bv", name="bv_row")
        nc.sync.dma_start(bv_row[:], bv_d[:])
        bv_r = cpool.tile([1, D], F32R, tag="bvr", name="bv_r")
        nc.vector.tensor_copy(bv_r[:], bv_row[:])
        nc.sync.dma_start(wv_sb[:], wv_d.rearrange("(t p) d -> p t d", p=P))
        for g in range(1, MT // GRP):
            emit_xkv_group(g)

        # ---------------- prefix: A, u, then per-half xqT -> q2 ------------
        with tc.tile_pool(name=f"pf{rep}", bufs=2, space="PSUM") as pfp:
            # A = Wq^T @ Wk first: needs only wq/wk, which land before x_q
            for dqt in range(KD):
                aps = pfp.tile([P, D], FP32, tag="pf1", name="aps")
                for ot in range(KD):
                    nc.tensor.matmul(aps[:],
                                     wq_sb[:, ot, dqt * P:(dqt + 1) * P],
                                     wk_sb[:, ot, :],
                                     start=(ot == 0), stop=(ot == KD - 1))
                nc.scalar.copy(A_sb[:, dqt, :], aps[:])

            # u = Wk^T @ bq  (row layout [1, dk])
            ups = pfp.tile([P, D], FP32, tag="pf1", name="ups")
            for ot in range(KD):
                nc.tensor.matmul(ups[0:1, :],
                                 bq_sb[:, ot, :],
                                 wk_sb[:, ot, :],
                                 start=(ot == 0), stop=(ot == KD - 1))
            nc.scalar.copy(u_row[:], ups[0:1, :])

            # per n-half: x_q transposes then q2^T = A^T-proj (+u via K=1
            # ones matmul), stored fp8 — the hh0 chain completes as soon as
            # the first x_q half-DMA lands, unblocking pass1/exp early
            for hh in range(2):
                for g in range(2 * hh, 2 * hh + 2):
                    qtp = pfp.tile([P, 2 * KD, P], FP32, tag="qtp",
                                   name="qtp")
                    for t in range(2):
                        nc_ = g * 2 + t
                        for a in range(KD):
                            nc.tensor.transpose(qtp[:, a * 2 + t, :],
                                                xqs[:, nc_, a * P:(a + 1) * P],
                                                ident[:])
                    for a in range(KD):
                        nc.scalar.copy(
                            xqT[:, a, g * 2 * P:(g + 1) * 2 * P],
                            qtp[:, a * 2:(a + 1) * 2, :])
                for dk in range(KD):
                    q2ps = pfp.tile([P, NH], FP32, tag="q2ps", name="q2ps")
                    for dqt in range(KD):
                        nc.tensor.matmul(q2ps[:],
                                         A_sb[:, dqt, dk * P:(dk + 1) * P],
                                         xqT[:, dqt, hh * NH:(hh + 1) * NH],
                                         start=(dqt == 0), stop=False)
                    nc.tensor.matmul(q2ps[:],
                                     u_row[0:1, dk * P:(dk + 1) * P],
                                     onesr[:],
                                     start=False, stop=True)
                    nc.scalar.copy(q2x[:, dk, hh * NH:(hh + 1) * NH],
                                   q2ps[:])

            # WvT via fp32 PE transposes (after q2 — not pass1-critical)
            for dt_ in range(KD):
                wtp = pfp.tile([P, D], FP32, tag="pf1", name="wtp")
                for ot in range(KD):
                    nc.tensor.transpose(wtp[:, ot * P:(ot + 1) * P],
                                        wv_sb[:, ot, dt_ * P:(dt_ + 1) * P],
                                        ident[:])
                nc.scalar.copy(WvT[:, dt_, :], wtp[:])

        pwp.release()
        if cut < 99:
            for dk in range(KD):
                for hh in range(2):
                    probes.append(q2x[:, dk, hh * NH:hh * NH + 1])
                probes.append(WvT[:, dk, 0:1])
        if cut < 2:
            _emit_probes(tc, fin, probes, out_d)
            return

        # ---------------- fused attention loop ----------------
        # psum banks: tp 1 + st 2x2 + o(hh0) 2 + z 1 = 8
        TG = 2                 # chunks per transpose group (= one pair)
        with tc.tile_pool(name=f"p8_{rep}", bufs=NPAIR) as p8p:
            tpp = tc.alloc_tile_pool(name=f"tp{rep}", bufs=1, space="PSUM")
            stp = tc.alloc_tile_pool(name=f"st{rep}", bufs=2, space="PSUM")
            oap = tc.alloc_tile_pool(name=f"oa{rep}", bufs=1, space="PSUM")
            zzp = tc.alloc_tile_pool(name=f"zz{rep}", bufs=1, space="PSUM")

            p8t = [p8p.tile([P, 2, 2, NH], FP8, tag="p8", name=f"p8_{gp}")
                   for gp in range(NPAIR)]
            o_ps = oap.tile([P, KD, NH], FP32, tag="oa", name="o_ps")
            if zsplit:
                zps = zzp.tile([1, 2, NH], FP32, tag="zz", name="zps")
            else:
                zps = zzp.tile([2, NH], FP32, tag="zz", name="zps")

            def transpose_group(tg):
                if notr:
                    return
                if dmat:
                    # bf16 DMA-transposes (XBAR) — no PE involvement at all;
                    # DVE cast bf16 -> fp8 afterwards
                    xT16 = xst.tile([P, KD, TG * P], BF16, tag="xT16",
                                    name="xT16")
                    for t in range(TG):
                        mi_ = tg * TG + t
                        for a in range(KD):
                            nc.sync.dma_start_transpose(
                                xT16[:, a, t * P:(t + 1) * P],
                                xkv16[:, mi_, a * P:(a + 1) * P])
                    for a in range(KD):
                        nc.vector.tensor_copy(
                            xkvT8[:, a, tg * TG * P:(tg + 1) * TG * P],
                            xT16[:, a, :])
                    return
                # fp32 PE transposes straight from the DMA staging tiles
                # (no Pool-convert dependency); DVE copy-out casts to fp8
                xsrc = xst_tiles[tg * TG // GRP]
                toff = (tg * TG) % GRP
                tp = tpp.tile([P, KD * TG, P], FP32, tag="tp", name="tp")
                for t in range(TG):
                    for a in range(KD):
                        # slab order (a, t): copy-out per a is contiguous
                        nc.tensor.transpose(
                            tp[:, a * TG + t, :],
                            xsrc[:, toff + t, a * P:(a + 1) * P],
                            ident[:])
                for a in range(KD):
                    nc.vector.tensor_copy(
                        xkvT8[:, a, tg * TG * P:(tg + 1) * TG * P],
                        tp[:, a * TG:(a + 1) * TG, :])

            transpose_group(0)
            for mi in range(MT):
                # one-pair lookahead: the DVE copy-out of group tg+1 runs
                # under the pass1/O matmuls of pair tg
                if mi % TG == 0 and mi // TG + 1 < NPAIR:
                    transpose_group(mi // TG + 1)

                # pass1: S^T chunk (one DR matmul per n-half, K=256)
                st = stp.tile([P, 2, NH], FP32, tag="st", name="st")
                for hh in range(2):
                    if q16:
                        for a in range(KD):
                            nc.tensor.matmul(
                                st[:, hh, :],
                                xkvT8[:, a, mi * P:(mi + 1) * P],
                                q2x[:, a, hh * NH:(hh + 1) * NH],
                                start=(a == 0), stop=(a == KD - 1),
                                skip_group_check=True)
                    else:
                        nc.tensor.matmul(
                            st[:, hh, :],
                            xkvT8[:, :, mi * P:(mi + 1) * P],
                            q2x[:, :, hh * NH:(hh + 1) * NH],
                            start=True, stop=True,
                            perf_mode=DR, skip_group_check=True)
                if mi < 2:
                    # first pair: per-half exps so ACT starts on the hh0
                    # result before the second x_q half-chain completes
                    for hh in range(2):
                        nc.scalar.activation(p8t[mi // 2][:, mi % 2, hh, :],
                                             st[:, hh, :], AF.Exp,
                                             scale=SCALE)
                else:
                    nc.scalar.activation(p8t[mi // 2][:, mi % 2, :, :],
                                         st[:], AF.Exp, scale=SCALE)

                if mi % 2 == 1:
                    gp = mi // 2
                    # O^T(hh0) DR accumulation (xkv8 pair stationary)
                    for db in range(KD):
                        nc.tensor.matmul(
                            o_ps[:, db, :],
                            xkv8[:, 2 * gp:2 * gp + 2, db * P:(db + 1) * P],
                            p8t[gp][:, :, 0, :],
                            start=(gp == 0), stop=(gp == NPAIR - 1),
                            perf_mode=DR, skip_group_check=True)
                    # Z for both halves into one bank: hh1 chain writes
                    # partitions {0,1} via the 0/1 selector (first MM of the
                    # bank, start=True); everything after accumulates.
                    if zsplit:
                        for hh in range(2):
                            nc.tensor.matmul(
                                zps[0:1, hh, :],
                                ones8[:, :, 0:1],
                                p8t[gp][:, :, hh, :],
                                start=(gp == 0), stop=(gp == NPAIR - 1),
                                perf_mode=DR, skip_group_check=True)
                    else:
                        nc.tensor.matmul(
                            zps[0:2, :],
                            zsel8[:, :, 2:4],
                            p8t[gp][:, :, 1, :],
                            start=(gp == 0), stop=False,
                            perf_mode=DR, skip_group_check=True)
                        nc.tensor.matmul(
                            zps[0:1, :],
                            ones8[:, :, 0:1],
                            p8t[gp][:, :, 0, :],
                            start=False, stop=(gp == NPAIR - 1),
                            perf_mode=DR, skip_group_check=True)

            if cut < 3:
                for gp in range(NPAIR):
                    for par in range(2):
                        probes.append(p8t[gp][:, par, 0, 0:1])
                _emit_probes(tc, fin, probes, out_d)
                return

            # ---- tail: evacuate o(hh0)/Z, reciprocal chain, O^T(hh1) ----
            for db in range(KD):
                nc.vector.tensor_copy(OT_sb[:, db, 0:NH], o_ps[:, db, :])
            if zsplit:
                nc.vector.tensor_copy(Z2[0:1, :], zps[0:1, 0, :])
                nc.vector.tensor_copy(Z2[1:2, :], zps[0:1, 1, :])
            else:
                nc.vector.tensor_copy(Z2[:], zps[0:2, :])
            # partition-merge [2, NH] -> [1, NL] via SBUF->SBUF DMAs
            nc.sync.dma_start(Z32[0:1, 0:NH], Z2[0:1, :])
            nc.sync.dma_start(Z32[0:1, NH:NL], Z2[1:2, :])
            nc.vector.tensor_copy(Zr[:], Z32[:])

            zzp.release()
            oap.release()
            stp.release()
            tpp.release()

            with tc.tile_pool(name=f"ob{rep}", bufs=1, space="PSUM") as obp, \
                 tc.tile_pool(name=f"ep{rep}", bufs=2, space="PSUM") as epp:
                o2 = obp.tile([P, KD, NH], FP32, tag="ob", name="o2")

                def epi_chunk(c):
                    fo = epp.tile([P, D], FP32, tag="ep", name="fo")
                    for dt_ in range(KD):
                        nc.tensor.matmul(fo[:],
                                         OT_sb[:, dt_, c * P:(c + 1) * P],
                                         WvT[:, dt_, :],
                                         start=(dt_ == 0), stop=False,
                                         skip_group_check=True)
                    nc.tensor.matmul(fo[:],
                                     Zr[0:1, c * P:(c + 1) * P],
                                     bv_r[:],
                                     start=False, stop=True,
                                     skip_group_check=True)
                    ob = fin.tile([P, D], FP32, tag="ob", name="ob")
                    nc.vector.tensor_scalar(ob[:], fo[:], rec_sb[:, c:c + 1],
                                            None, op0=mult)
                    nc.sync.dma_start(out_d[c * P:(c + 1) * P, :], ob[:])

                # O^T(hh1) DR stream with the reciprocal chain and the hh0
                # epilogue chunks interleaved
                for gp in range(NPAIR):
                    for db in range(KD):
                        nc.tensor.matmul(
                            o2[:, db, :],
                            xkv8[:, 2 * gp:2 * gp + 2, db * P:(db + 1) * P],
                            p8t[gp][:, :, 1, :],
                            start=(gp == 0), stop=(gp == NPAIR - 1),
                            perf_mode=DR, skip_group_check=True)
                    if gp == 7:
                        # 1/Z per n-chunk (transpose Z to partitions); the
                        # Z32 DMA has landed by now, so no PE stall
                        rt = epp.tile([P, NCH], FP32, tag="ep", name="rt")
                        for c in range(NCH):
                            nc.tensor.transpose(rt[:, c:c + 1],
                                                Z32[0:1, c * P:(c + 1) * P],
                                                ident[0:1, 0:1])
                        nc.vector.reciprocal(rec_sb[:], rt[:])
                    if cut >= 99 and gp in (15, 19, 23, 27):
                        epi_chunk((gp - 15) // 4)

                for db in range(KD):
                    nc.vector.tensor_copy(OT_sb[:, db, NH:NL], o2[:, db, :])

                if cut < 99:
                    for dk in range(KD):
                        for hh in range(2):
                            probes.append(OT_sb[:, dk, hh * NH:hh * NH + 1])
                    probes.append(Zr[0:1, 0:1])
                    probes.append(Z32[0:1, 0:1])
                    _emit_probes(tc, fin, probes, out_d)
                    return

                # ---- epilogue: out = (O^T.T @ Wv^T + Z*bv) / Z ----
                for c in range(4, NCH):
                    epi_chunk(c)


# ---------------------------------------------------------------------------
# host side: build once, run via a persistent sharded jit
# ---------------------------------------------------------------------------

_CACHE = {}


def _get_runner(reps=1, muls=()):
    if not muls:
        import json
        import os
        muls = json.loads(os.environ.get("KMULS", "{}"))
    muls = tuple(sorted(dict(muls).items()))
    key = f"runner{reps}_{muls}"
    if key in _CACHE:
        return _CACHE[key]

    import jax
    from jax.experimental.shard_map import shard_map
    from jax.sharding import Mesh, PartitionSpec

    from concourse import bass2jax
    from concourse.bass2jax import _bass_exec_p, install_neuronx_cc_hook

    install_neuronx_cc_hook()
    nc = _build_nc(reps=reps, muls=muls)

    partition_name = (nc.partition_id_tensor.name
                      if nc.partition_id_tensor else None)
    in_names, out_names, out_avals, zero_outs = [], [], [], []
    for alloc in nc.m.functions[0].allocations:
        if not isinstance(alloc, mybir.MemoryLocationSet):
            continue
        name = alloc.memorylocations[0].name
        if alloc.kind == "ExternalInput":
            if name != partition_name:
                in_names.append(name)
        elif alloc.kind == "ExternalOutput":
            shape = tuple(alloc.tensor_shape)
            dtype = mybir.dt.np(alloc.dtype)
            out_names.append(name)
            out_avals.append(jax.core.ShapedArray(shape, dtype))
            zero_outs.append(np.zeros(shape, dtype))
    n_params = len(in_names)
    all_in_names = list(in_names) + list(out_names)
    if partition_name is not None:
        all_in_names.append(partition_name)

    def _bodyfn(*args):
        operands = list(args)
        if partition_name is not None:
            operands.append(bass2jax.partition_id_tensor())
        outs = _bass_exec_p.bind(
            *operands,
            out_avals=tuple(out_avals),
            in_names=tuple(all_in_names),
            out_names=tuple(out_names),
            lowering_input_output_aliases=(),
            sim_require_finite=True,
            sim_require_nnan=True,
            nc=nc,
        )
        return tuple(outs)

    devices = jax.devices()[:NCORES]
    mesh = Mesh(np.asarray(devices), ("core",))
    n_outs = len(out_names)
    sharded = jax.jit(
        shard_map(_bodyfn, mesh=mesh,
                  in_specs=(PartitionSpec("core"),) * (n_params + n_outs),
                  out_specs=(PartitionSpec("core"),) * n_outs,
                  check_rep=False),
        keep_unused=True)

    runner = {
        "fn": sharded,
        "in_names": in_names,
        "out_names": out_names,
        "out_avals": out_avals,
        "zero_outs": zero_outs,
        "mesh": mesh,
    }
    _CACHE[key] = runner
    return runner


def make_core_inputs(x_q, x_kv, Wq, bq, Wk, bk, Wv, bv):
    """Shared per-core input dict (x_q handled per core)."""
    f32 = np.float32
    return {
        "x_kv": np.ascontiguousarray(x_kv, dtype=f32),
        "Wq": np.ascontiguousarray(Wq, dtype=f32),
        "Wk": np.ascontiguousarray(Wk, dtype=f32),
        "Wv": np.ascontiguousarray(Wv, dtype=f32),
        "bq": np.ascontiguousarray(np.asarray(bq, dtype=f32).reshape(KD, P, 1)),
        "bv": np.ascontiguousarray(np.asarray(bv, dtype=f32).reshape(1, D)),
    }


def _make_concat_inputs(x_q, x_kv, Wq, bq, Wk, bk, Wv, bv):
    per_core_shared = make_core_inputs(x_q, x_kv, Wq, bq, Wk, bk, Wv, bv)
    x_q = np.ascontiguousarray(x_q, dtype=np.float32)

    def core_input(name, c):
        if name == "x_q":
            return x_q[c * NL:(c + 1) * NL]
        return per_core_shared[name]

    runner = _get_runner()
    concat = []
    for name in runner["in_names"]:
        concat.append(np.concatenate(
            [core_input(name, c) for c in range(NCORES)], axis=0))
    return concat


def kernel(x_q, x_kv, Wq, bq, Wk, bk, Wv, bv):
    runner = _get_runner()
    concat_in = _make_concat_inputs(x_q, x_kv, Wq, bq, Wk, bk, Wv, bv)
    concat_zeros = [np.zeros((NCORES * z.shape[0], *z.shape[1:]), z.dtype)
                    for z in runner["zero_outs"]]
    outs = runner["fn"](*concat_in, *concat_zeros)
    idx = runner["out_names"].index("out")
    full = np.asarray(outs[idx])
    return full.astype(np.float32)
